# revision 25
# baseline (speedup 1.0000x reference)
"""Multi-head causal self-attention on 8 Trainium2 NeuronCores.

Problem: X[4,2048,1024], per-head Wq/Wk/Wv[16,1024,64], Wo[1024,1024], bo[1024].
    out = OutProj(concat_heads(softmax_causal(Q K^T / 8) V))

Sharding: 8 cores = 4 batches x 2 head-groups (8 heads each). Each core
computes its batch's attention for its 8 heads plus the partial output
projection over its 512 concat features; host sums the two partials per
batch and adds the bias.

Per-core kernel (matmul operands in fp16 — 1 cycle/row on TensorE and
fp32 PSUM accumulation; softmax runs in the transposed
"feature-on-partition" space so its reduction lands on the free dim):
  qT/kT per head-pair  [128, T]  = Wpair^T  x  X^T
  v    per s-tile      [128, 8*65] = X^T^T  x  Wv_all (65th col set to 1)
  ST block [s=128, t=512] = kT_slice^T @ qT_slice   (row-packed head pairs)
  expST = exp(ST/8) (ScalarE), causal-masked via gpsimd affine_select
  avT [65, 512] += [V|1]^T @ expST   -> rows 0:64 = (A@V)^T, row 64 = sums
  normalize via 1/sums broadcast (rank-1 fp32 matmul) and write concatT
  partial = concatT^T @ WoST  (accumulated over 4 feature chunks)
"""

import os
import sys

for _p in ("/opt/trn_rl_repo", "/root/.axon_site/_ro/trn_rl_repo"):
    if os.path.isdir(_p) and _p not in sys.path:
        sys.path.append(_p)

import numpy as np

import concourse.mybir as mybir
import concourse.tile as tile
from concourse import bacc

B, T, D, H, K = 4, 2048, 1024, 16, 64
HG = 8          # heads per core
NPAIR = 4       # head pairs per core
P = 128
DC = D // P     # 8 contraction chunks for the projections
NS = T // P     # 16 key tiles
NT = T // 512   # 4 query tiles of 512
F32 = mybir.dt.float32
F16 = mybir.dt.float16


def build_module():
    nc = bacc.Bacc("TRN2")
    XT = nc.dram_tensor("xt", [D, T], F16, kind="ExternalInput").ap()
    WQ = nc.dram_tensor("wq", [NPAIR, D, P], F16, kind="ExternalInput").ap()
    WK = nc.dram_tensor("wk", [NPAIR, D, P], F16, kind="ExternalInput").ap()
    WV = nc.dram_tensor("wv", [D, HG * K], F16, kind="ExternalInput").ap()
    WO = nc.dram_tensor("wo", [HG * K, D], F16, kind="ExternalInput").ap()
    OUT = nc.dram_tensor("out", [T, D], F16, kind="ExternalOutput").ap()

    with tile.TileContext(nc) as tc:
        with tc.tile_pool(name="persist", bufs=1) as pp:
            xt_sb = pp.tile([P, DC, T], F16)            # X^T, 32 KB/partition
            v_sb = pp.tile([P, NS, HG * (K + 1)], F16)  # V + ones col per head
            concat_sb = pp.tile([P, NPAIR, T], F16)     # concat(heads)^T
            tri_sb = pp.tile([P, P], F16)   # causal triangle: 1 where x >= p

            xt_r = XT.rearrange("(c p) t -> c p t", p=P)
            for c in range(DC):
                nc.sync.dma_start(out=xt_sb[:, c, :], in_=xt_r[c])
            wo_sb = pp.tile([P, NPAIR, D], F16)
            nc.sync.dma_start(
                out=wo_sb, in_=WO.rearrange("(s p) o -> p s o", p=P)
            )
            nc.vector.memset(tri_sb, 1.0)
            nc.gpsimd.affine_select(
                out=tri_sb,
                in_=tri_sb,
                compare_op=mybir.AluOpType.is_ge,
                fill=0.0,
                base=0,
                channel_multiplier=-1,
                pattern=[[1, P]],
            )
            # ones column (index 64 of each head's 65-wide slot)
            v_slots = v_sb.rearrange("p s (h x) -> p s h x", x=K + 1)
            nc.vector.memset(v_slots[:, :, :, K : K + 1], 1.0)

            # ---- V + Q/K projections + attention, software-pipelined ----
            # Projection matmuls for V (tail) and the NEXT pair's Q/K are
            # interleaved into the attention loop so the PE fills the
            # stalls where it would otherwise wait on ScalarE's exp.
            with (
                tc.tile_pool(name="wvp", bufs=1) as wvp,
                tc.tile_pool(name="attn", bufs=1) as ap_,
                tc.tile_pool(name="psa", bufs=1, space="PSUM") as psa,
            ):
                wv_sb = wvp.tile([P, DC, HG * K], F16)
                nc.sync.dma_start(
                    out=wv_sb, in_=WV.rearrange("(c p) n -> p c n", p=P)
                )

                def v_group_ops(s):
                    ps = psa.tile(
                        [P, HG * K], F32, tag="mm", bufs=1, name=f"vps{s}"
                    )
                    ops = [
                        (
                            lambda c=c, ps=ps, s=s: nc.tensor.matmul(
                                ps,
                                xt_sb[:, c, s * P : (s + 1) * P],
                                wv_sb[:, c, :],
                                start=(c == 0),
                                stop=(c == DC - 1),
                            )
                        )
                        for c in range(DC)
                    ]
                    ops.append(
                        lambda ps=ps, s=s: nc.vector.tensor_copy(
                            v_slots[:, s, :, 0:K],
                            ps.rearrange("p (h k) -> p h k", k=K),
                        )
                    )
                    return ops

                def proj_pair(pr):
                    """DMA the pair's weights; return (q, k, deferred ops)."""
                    wq_sb = ap_.tile(
                        [P, DC, P], F16, tag="wq", bufs=2, name=f"wq{pr}"
                    )
                    wk_sb = ap_.tile(
                        [P, DC, P], F16, tag="wk", bufs=2, name=f"wk{pr}"
                    )
                    nc.sync.dma_start(
                        out=wq_sb, in_=WQ[pr].rearrange("(c p) m -> p c m", p=P)
                    )
                    nc.sync.dma_start(
                        out=wk_sb, in_=WK[pr].rearrange("(c p) m -> p c m", p=P)
                    )
                    q_sb = ap_.tile([P, T], F16, tag="q", bufs=2, name=f"q{pr}")
                    k_sb = ap_.tile([P, T], F16, tag="k", bufs=2, name=f"k{pr}")
                    ops = []
                    for w_sb, qk_sb, nm in (
                        (wq_sb, q_sb, "q"),
                        (wk_sb, k_sb, "k"),
                    ):
                        for tt in range(NT):
                            ps = psa.tile(
                                [P, 512], F32, tag="mm", bufs=1,
                                name=f"{nm}ps{pr}_{tt}",
                            )
                            for c in range(DC):
                                ops.append(
                                    lambda ps=ps, w_sb=w_sb, c=c, tt=tt:
                                    nc.tensor.matmul(
                                        ps,
                                        w_sb[:, c, :],
                                        xt_sb[
                                            :, c, tt * 512 : (tt + 1) * 512
                                        ],
                                        start=(c == 0),
                                        stop=(c == DC - 1),
                                    )
                                )
                            ops.append(
                                lambda ps=ps, qk_sb=qk_sb, tt=tt:
                                nc.vector.tensor_copy(
                                    qk_sb[:, tt * 512 : (tt + 1) * 512], ps
                                )
                            )
                    return q_sb, k_sb, ops

                # upfront: V for the first four key tiles + pair 0's Q/K
                for s in range(4):
                    for op in v_group_ops(s):
                        op()
                q_sb, k_sb, ops0 = proj_pair(0)
                for op in ops0:
                    op()
                vqueue = [op for s in range(4, NS) for op in v_group_ops(s)]
                pending = []
                v_done = [0]

                for pr in range(NPAIR):
                    if pr < NPAIR - 1:
                        nq_sb, nk_sb, nops = proj_pair(pr + 1)
                        pending.extend(nops)

                    for tt in range(NT):
                        if pr == 0:
                            # V for this tt's key tiles must be in flight
                            need = (4 * tt) * 9
                            while v_done[0] < need and vqueue:
                                vqueue.pop(0)()
                                v_done[0] += 1
                        avs = [
                            psa.tile(
                                [K + 1, 512], F32, tag="av", bufs=3,
                                name=f"av{pr}_{tt}_{h2}",
                            )
                            for h2 in range(2)
                        ]
                        n_s = 4 * tt + 4
                        for si in range(n_s):
                            for _ in range(4):
                                if vqueue:
                                    vqueue.pop(0)()
                                    v_done[0] += 1
                                elif pending:
                                    pending.pop(0)()
                            # diagonal blocks: only cols >= 128*m can be valid
                            m = si - 4 * tt
                            off = max(m, 0) * P
                            nv = 512 - off
                            # both heads' score blocks in one 2-bank tile
                            st = psa.tile([P, 2, 512], F32, tag="stw", bufs=2)
                            ex = ap_.tile([P, 2, 512], F16, tag="exp", bufs=6)
                            for h in range(2):
                                lo, hi = h * K, (h + 1) * K
                                nc.tensor.matmul(
                                    st[:, h, 0:nv],
                                    k_sb[lo:hi, si * P : (si + 1) * P],
                                    q_sb[
                                        lo:hi,
                                        tt * 512 + off : (tt + 1) * 512,
                                    ],
                                    start=True,
                                    stop=True,
                                    tile_position=(lo, 0),
                                )
                            nc.scalar.activation(
                                ex[:, :, 0:nv], st[:, :, 0:nv],
                                mybir.ActivationFunctionType.Exp,
                                scale=0.125,
                            )
                            if m >= 0:  # mask both heads' leading triangles
                                nc.vector.tensor_mul(
                                    ex[:, :, 0:P],
                                    ex[:, :, 0:P],
                                    tri_sb.unsqueeze(1).broadcast_to(
                                        [P, 2, P]
                                    ),
                                )
                            for h in range(2):
                                slot = (2 * pr + h) * (K + 1)
                                nc.tensor.matmul(
                                    avs[h][:, off:512],
                                    v_sb[:, si, slot : slot + K + 1],
                                    ex[:, h, 0:nv],
                                    start=(si == 0),
                                    stop=(si == n_s - 1),
                                )
                        # Free the AV accumulators fast: copy raw (A@V)^T and
                        # the sums row out, then normalize in place off the
                        # critical path (divide happens before outproj reads).
                        for h in range(2):
                            cols = slice(tt * 512, (tt + 1) * 512)
                            sums = ap_.tile([1, 512], F32, tag="sums", bufs=4)
                            nc.vector.tensor_copy(sums, avs[h][K : K + 1, :])
                            if h == 0:
                                dst = concat_sb[0:K, pr, cols]
                                nc.vector.tensor_copy(dst, avs[h][0:K, :])
                            else:
                                dst = ap_.tile([K, 512], F16, tag="tmpb", bufs=4)
                                nc.vector.tensor_copy(dst, avs[h][0:K, :])
                            recip = ap_.tile([1, 512], F32, tag="recip", bufs=4)
                            nc.vector.reciprocal_approx_fast(recip, sums)
                            bc_sb = ap_.tile([K, 512], F32, tag="bc_sb", bufs=4)
                            nc.gpsimd.partition_broadcast(bc_sb, recip)
                            nc.vector.tensor_mul(dst, dst, bc_sb)
                            if h == 1:
                                # partition-shifted write via DMA bounce
                                nc.sync.dma_start(
                                    out=concat_sb[K:P, pr, cols], in_=dst
                                )

                    # next pair's projections must be complete before its
                    # attention starts; flush whatever wasn't interleaved
                    while pending:
                        pending.pop(0)()
                    if pr < NPAIR - 1:
                        q_sb, k_sb = nq_sb, nk_sb

            # ---- output projection (partial: this core's 512 features) ----
            with (
                tc.tile_pool(name="op", bufs=1) as op_,
                tc.tile_pool(name="pso", bufs=1, space="PSUM") as pso,
            ):
                for t16 in range(T // P):
                    for oc in range(2):
                        ps = pso.tile([P, 512], F32, tag="mm", bufs=4)
                        for s4 in range(NPAIR):
                            nc.tensor.matmul(
                                ps,
                                concat_sb[:, s4, t16 * P : (t16 + 1) * P],
                                wo_sb[:, s4, oc * 512 : (oc + 1) * 512],
                                start=(s4 == 0),
                                stop=(s4 == NPAIR - 1),
                            )
                        st_o = op_.tile([P, 512], F16, tag="outst", bufs=3)
                        nc.vector.tensor_copy(st_o, ps)
                        nc.sync.dma_start(
                            out=OUT[
                                t16 * P : (t16 + 1) * P, oc * 512 : (oc + 1) * 512
                            ],
                            in_=st_o,
                        )
    nc.compile()
    return nc


def shard_inputs(X, Wq, Wk, Wv, Wo):
    """Host-side shard prep: core c handles batch c//2, head group c%2."""
    in_maps = []
    for c in range(8):
        b, g = c // 2, c % 2
        heads = range(g * HG, (g + 1) * HG)
        wq = np.stack(
            [
                np.concatenate([Wq[g * HG + 2 * p], Wq[g * HG + 2 * p + 1]], axis=1)
                for p in range(NPAIR)
            ]
        )
        wk = np.stack(
            [
                np.concatenate([Wk[g * HG + 2 * p], Wk[g * HG + 2 * p + 1]], axis=1)
                for p in range(NPAIR)
            ]
        )
        wv = np.concatenate([Wv[h] for h in heads], axis=1)
        wo = Wo[:, g * 512 : (g + 1) * 512].T
        in_maps.append(
            {
                "xt": np.ascontiguousarray(X[b].T).astype(np.float16),
                "wq": np.ascontiguousarray(wq).astype(np.float16),
                "wk": np.ascontiguousarray(wk).astype(np.float16),
                "wv": np.ascontiguousarray(wv).astype(np.float16),
                "wo": np.ascontiguousarray(wo).astype(np.float16),
            }
        )
    return in_maps


_MODULE = None


def _get_module():
    global _MODULE
    if _MODULE is None:
        _MODULE = build_module()
    return _MODULE


def kernel(X, Wq, Wk, Wv, Wo, bo, _want_results=None):
    from concourse.bass_utils import run_bass_kernel_spmd

    nc = _get_module()
    in_maps = shard_inputs(
        np.asarray(X), np.asarray(Wq), np.asarray(Wk), np.asarray(Wv), np.asarray(Wo)
    )
    res = run_bass_kernel_spmd(nc, in_maps, core_ids=list(range(8)))
    if _want_results is not None:
        _want_results.append(res)
    out = np.empty((B, T, H * K), dtype=np.float32)
    bo = np.asarray(bo, dtype=np.float32)
    for b in range(B):
        out[b] = (
            res.results[2 * b]["out"].astype(np.float32)
            + res.results[2 * b + 1]["out"].astype(np.float32)
            + bo
        )
    return out


# revision 26
# speedup vs baseline: 1.0695x; 1.0695x over previous
"""Multi-head causal self-attention on 8 Trainium2 NeuronCores.

Problem: X[4,2048,1024], per-head Wq/Wk/Wv[16,1024,64], Wo[1024,1024], bo[1024].
    out = OutProj(concat_heads(softmax_causal(Q K^T / 8) V))

Sharding: 8 cores = 4 batches x 2 head-groups (8 heads each). Each core
computes its batch's attention for its 8 heads plus the partial output
projection over its 512 concat features; host sums the two partials per
batch and adds the bias.

Per-core kernel (matmul operands in fp16 — 1 cycle/row on TensorE and
fp32 PSUM accumulation; softmax runs in the transposed
"feature-on-partition" space so its reduction lands on the free dim):
  qT/kT per head-pair  [128, T]  = Wpair^T  x  X^T
  v    per s-tile      [128, 8*65] = X^T^T  x  Wv_all (65th col set to 1)
  ST block [s=128, t=512] = kT_slice^T @ qT_slice   (row-packed head pairs)
  expST = exp(ST/8) (ScalarE), causal-masked via gpsimd affine_select
  avT [65, 512] += [V|1]^T @ expST   -> rows 0:64 = (A@V)^T, row 64 = sums
  normalize via 1/sums broadcast (rank-1 fp32 matmul) and write concatT
  partial = concatT^T @ WoST  (accumulated over 4 feature chunks)
"""

import os
import sys

for _p in ("/opt/trn_rl_repo", "/root/.axon_site/_ro/trn_rl_repo"):
    if os.path.isdir(_p) and _p not in sys.path:
        sys.path.append(_p)

import numpy as np

import concourse.mybir as mybir
import concourse.tile as tile
from concourse import bacc

B, T, D, H, K = 4, 2048, 1024, 16, 64
HG = 8          # heads per core
NPAIR = 4       # head pairs per core
P = 128
DC = D // P     # 8 contraction chunks for the projections
NS = T // P     # 16 key tiles
NT = T // 512   # 4 query tiles of 512
F32 = mybir.dt.float32
F16 = mybir.dt.float16


def build_module():
    nc = bacc.Bacc("TRN2")
    XT = nc.dram_tensor("xt", [D, T], F16, kind="ExternalInput").ap()
    WQ = nc.dram_tensor("wq", [NPAIR, D, P], F16, kind="ExternalInput").ap()
    WK = nc.dram_tensor("wk", [NPAIR, D, P], F16, kind="ExternalInput").ap()
    WV = nc.dram_tensor("wv", [D, HG * K], F16, kind="ExternalInput").ap()
    WO = nc.dram_tensor("wo", [HG * K, D], F16, kind="ExternalInput").ap()
    OUT = nc.dram_tensor("out", [T, D], F16, kind="ExternalOutput").ap()

    with tile.TileContext(nc) as tc:
        with tc.tile_pool(name="persist", bufs=1) as pp:
            xt_sb = pp.tile([P, DC, T], F16)            # X^T, 32 KB/partition
            v_sb = pp.tile([P, NS, HG * (K + 1)], F16)  # V + ones col per head
            concat_sb = pp.tile([P, NPAIR, T], F16)     # concat(heads)^T
            tri_sb = pp.tile([P, P], F16)   # causal triangle: 1 where x >= p

            xt_r = XT.rearrange("(c p) t -> c p t", p=P)
            for c in range(DC):
                nc.sync.dma_start(out=xt_sb[:, c, :], in_=xt_r[c])
            wo_sb = pp.tile([P, NPAIR, D], F16)
            nc.sync.dma_start(
                out=wo_sb, in_=WO.rearrange("(s p) o -> p s o", p=P)
            )
            nc.vector.memset(tri_sb, 1.0)
            nc.gpsimd.affine_select(
                out=tri_sb,
                in_=tri_sb,
                compare_op=mybir.AluOpType.is_ge,
                fill=0.0,
                base=0,
                channel_multiplier=-1,
                pattern=[[1, P]],
            )
            # ones column (index 64 of each head's 65-wide slot)
            v_slots = v_sb.rearrange("p s (h x) -> p s h x", x=K + 1)
            nc.vector.memset(v_slots[:, :, :, K : K + 1], 1.0)

            # ---- V + Q/K projections + attention, software-pipelined ----
            # Projection matmuls for V (tail) and the NEXT pair's Q/K are
            # interleaved into the attention loop so the PE fills the
            # stalls where it would otherwise wait on ScalarE's exp.
            with (
                tc.tile_pool(name="wvp", bufs=1) as wvp,
                tc.tile_pool(name="attn", bufs=1) as ap_,
                tc.tile_pool(name="psa", bufs=1, space="PSUM") as psa,
            ):
                wv_sb = wvp.tile([P, DC, HG * K], F16)
                nc.sync.dma_start(
                    out=wv_sb, in_=WV.rearrange("(c p) n -> p c n", p=P)
                )

                def v_group_ops(s):
                    ps = psa.tile(
                        [P, HG * K], F32, tag="mm", bufs=2, name=f"vps{s}"
                    )
                    ops = [
                        (
                            lambda c=c, ps=ps, s=s: nc.tensor.matmul(
                                ps,
                                xt_sb[:, c, s * P : (s + 1) * P],
                                wv_sb[:, c, :],
                                start=(c == 0),
                                stop=(c == DC - 1),
                            )
                        )
                        for c in range(DC)
                    ]
                    ops.append(
                        lambda ps=ps, s=s: nc.vector.tensor_copy(
                            v_slots[:, s, :, 0:K],
                            ps.rearrange("p (h k) -> p h k", k=K),
                        )
                    )
                    return ops

                def proj_pair(pr):
                    """DMA the pair's weights; return (q, k, deferred ops)."""
                    wq_sb = ap_.tile(
                        [P, DC, P], F16, tag="wq", bufs=2, name=f"wq{pr}"
                    )
                    wk_sb = ap_.tile(
                        [P, DC, P], F16, tag="wk", bufs=2, name=f"wk{pr}"
                    )
                    nc.sync.dma_start(
                        out=wq_sb, in_=WQ[pr].rearrange("(c p) m -> p c m", p=P)
                    )
                    nc.sync.dma_start(
                        out=wk_sb, in_=WK[pr].rearrange("(c p) m -> p c m", p=P)
                    )
                    q_sb = ap_.tile([P, T], F16, tag="q", bufs=2, name=f"q{pr}")
                    k_sb = ap_.tile([P, T], F16, tag="k", bufs=2, name=f"k{pr}")
                    ops = []
                    for w_sb, qk_sb, nm in (
                        (wq_sb, q_sb, "q"),
                        (wk_sb, k_sb, "k"),
                    ):
                        for tt in range(NT):
                            ps = psa.tile(
                                [P, 512], F32, tag="mm", bufs=2,
                                name=f"{nm}ps{pr}_{tt}",
                            )
                            for c in range(DC):
                                ops.append(
                                    lambda ps=ps, w_sb=w_sb, c=c, tt=tt:
                                    nc.tensor.matmul(
                                        ps,
                                        w_sb[:, c, :],
                                        xt_sb[
                                            :, c, tt * 512 : (tt + 1) * 512
                                        ],
                                        start=(c == 0),
                                        stop=(c == DC - 1),
                                    )
                                )
                            ops.append(
                                lambda ps=ps, qk_sb=qk_sb, tt=tt:
                                nc.vector.tensor_copy(
                                    qk_sb[:, tt * 512 : (tt + 1) * 512], ps
                                )
                            )
                    return q_sb, k_sb, ops

                # upfront: V for the first four key tiles + pair 0's Q/K
                for s in range(4):
                    for op in v_group_ops(s):
                        op()
                q_sb, k_sb, ops0 = proj_pair(0)
                for op in ops0:
                    op()
                vqueue = [op for s in range(4, NS) for op in v_group_ops(s)]
                pending = []
                v_done = [0]

                for pr in range(NPAIR):
                    if pr < NPAIR - 1:
                        nq_sb, nk_sb, nops = proj_pair(pr + 1)
                        pending.extend(nops)

                    for tt in range(NT):
                        if pr == 0:
                            # V for this tt's key tiles must be in flight
                            need = (4 * tt) * 9
                            while v_done[0] < need and vqueue:
                                vqueue.pop(0)()
                                v_done[0] += 1
                        avs = [
                            psa.tile(
                                [K + 1, 512], F32, tag="av", bufs=2,
                                name=f"av{pr}_{tt}_{h2}",
                            )
                            for h2 in range(2)
                        ]
                        n_s = 4 * tt + 4
                        for si in range(n_s):
                            for _ in range(4):
                                if vqueue:
                                    vqueue.pop(0)()
                                    v_done[0] += 1
                                elif pending:
                                    pending.pop(0)()
                            # diagonal blocks: only cols >= 128*m can be valid
                            m = si - 4 * tt
                            off = max(m, 0) * P
                            nv = 512 - off
                            # both heads' score blocks in one 2-bank tile
                            st = psa.tile([P, 2, 512], F32, tag="stw", bufs=2)
                            ex = ap_.tile([P, 2, 512], F16, tag="exp", bufs=6)
                            for h in range(2):
                                lo, hi = h * K, (h + 1) * K
                                nc.tensor.matmul(
                                    st[:, h, 0:nv],
                                    k_sb[lo:hi, si * P : (si + 1) * P],
                                    q_sb[
                                        lo:hi,
                                        tt * 512 + off : (tt + 1) * 512,
                                    ],
                                    start=True,
                                    stop=True,
                                    tile_position=(lo, 0),
                                )
                            nc.scalar.activation(
                                ex[:, :, 0:nv], st[:, :, 0:nv],
                                mybir.ActivationFunctionType.Exp,
                                scale=0.125,
                            )
                            if m >= 0:  # mask both heads' leading triangles
                                nc.vector.tensor_mul(
                                    ex[:, :, 0:P],
                                    ex[:, :, 0:P],
                                    tri_sb.unsqueeze(1).broadcast_to(
                                        [P, 2, P]
                                    ),
                                )
                            for h in range(2):
                                slot = (2 * pr + h) * (K + 1)
                                nc.tensor.matmul(
                                    avs[h][:, off:512],
                                    v_sb[:, si, slot : slot + K + 1],
                                    ex[:, h, 0:nv],
                                    start=(si == 0),
                                    stop=(si == n_s - 1),
                                )
                        # Free the AV accumulators fast: copy raw (A@V)^T and
                        # the sums row out, then normalize in place off the
                        # critical path (divide happens before outproj reads).
                        for h in range(2):
                            cols = slice(tt * 512, (tt + 1) * 512)
                            sums = ap_.tile([1, 512], F32, tag="sums", bufs=4)
                            nc.vector.tensor_copy(sums, avs[h][K : K + 1, :])
                            if h == 0:
                                dst = concat_sb[0:K, pr, cols]
                                nc.vector.tensor_copy(dst, avs[h][0:K, :])
                            else:
                                dst = ap_.tile([K, 512], F16, tag="tmpb", bufs=4)
                                nc.vector.tensor_copy(dst, avs[h][0:K, :])
                            recip = ap_.tile([1, 512], F32, tag="recip", bufs=4)
                            nc.vector.reciprocal_approx_fast(recip, sums)
                            bc_sb = ap_.tile([K, 512], F32, tag="bc_sb", bufs=4)
                            nc.gpsimd.partition_broadcast(bc_sb, recip)
                            nc.vector.tensor_mul(dst, dst, bc_sb)
                            if h == 1:
                                # partition-shifted write via DMA bounce
                                nc.sync.dma_start(
                                    out=concat_sb[K:P, pr, cols], in_=dst
                                )

                    # next pair's projections must be complete before its
                    # attention starts; flush whatever wasn't interleaved
                    while pending:
                        pending.pop(0)()
                    if pr < NPAIR - 1:
                        q_sb, k_sb = nq_sb, nk_sb

            # ---- output projection (partial: this core's 512 features) ----
            with (
                tc.tile_pool(name="op", bufs=1) as op_,
                tc.tile_pool(name="pso", bufs=1, space="PSUM") as pso,
            ):
                for t16 in range(T // P):
                    for oc in range(2):
                        ps = pso.tile([P, 512], F32, tag="mm", bufs=4)
                        for s4 in range(NPAIR):
                            nc.tensor.matmul(
                                ps,
                                concat_sb[:, s4, t16 * P : (t16 + 1) * P],
                                wo_sb[:, s4, oc * 512 : (oc + 1) * 512],
                                start=(s4 == 0),
                                stop=(s4 == NPAIR - 1),
                            )
                        st_o = op_.tile([P, 512], F16, tag="outst", bufs=3)
                        nc.vector.tensor_copy(st_o, ps)
                        nc.sync.dma_start(
                            out=OUT[
                                t16 * P : (t16 + 1) * P, oc * 512 : (oc + 1) * 512
                            ],
                            in_=st_o,
                        )
    nc.compile()
    return nc


def shard_inputs(X, Wq, Wk, Wv, Wo):
    """Host-side shard prep: core c handles batch c//2, head group c%2."""
    in_maps = []
    for c in range(8):
        b, g = c // 2, c % 2
        heads = range(g * HG, (g + 1) * HG)
        wq = np.stack(
            [
                np.concatenate([Wq[g * HG + 2 * p], Wq[g * HG + 2 * p + 1]], axis=1)
                for p in range(NPAIR)
            ]
        )
        wk = np.stack(
            [
                np.concatenate([Wk[g * HG + 2 * p], Wk[g * HG + 2 * p + 1]], axis=1)
                for p in range(NPAIR)
            ]
        )
        wv = np.concatenate([Wv[h] for h in heads], axis=1)
        wo = Wo[:, g * 512 : (g + 1) * 512].T
        in_maps.append(
            {
                "xt": np.ascontiguousarray(X[b].T).astype(np.float16),
                "wq": np.ascontiguousarray(wq).astype(np.float16),
                "wk": np.ascontiguousarray(wk).astype(np.float16),
                "wv": np.ascontiguousarray(wv).astype(np.float16),
                "wo": np.ascontiguousarray(wo).astype(np.float16),
            }
        )
    return in_maps


_MODULE = None


def _get_module():
    global _MODULE
    if _MODULE is None:
        _MODULE = build_module()
    return _MODULE


def kernel(X, Wq, Wk, Wv, Wo, bo, _want_results=None):
    from concourse.bass_utils import run_bass_kernel_spmd

    nc = _get_module()
    in_maps = shard_inputs(
        np.asarray(X), np.asarray(Wq), np.asarray(Wk), np.asarray(Wv), np.asarray(Wo)
    )
    res = run_bass_kernel_spmd(nc, in_maps, core_ids=list(range(8)))
    if _want_results is not None:
        _want_results.append(res)
    out = np.empty((B, T, H * K), dtype=np.float32)
    bo = np.asarray(bo, dtype=np.float32)
    for b in range(B):
        out[b] = (
            res.results[2 * b]["out"].astype(np.float32)
            + res.results[2 * b + 1]["out"].astype(np.float32)
            + bo
        )
    return out


# revision 27
# speedup vs baseline: 1.0950x; 1.0238x over previous
"""Multi-head causal self-attention on 8 Trainium2 NeuronCores.

Problem: X[4,2048,1024], per-head Wq/Wk/Wv[16,1024,64], Wo[1024,1024], bo[1024].
    out = OutProj(concat_heads(softmax_causal(Q K^T / 8) V))

Sharding: 8 cores = 4 batches x 2 head-groups (8 heads each). Each core
computes its batch's attention for its 8 heads plus the partial output
projection over its 512 concat features; host sums the two partials per
batch and adds the bias.

Per-core kernel (matmul operands in fp16 — 1 cycle/row on TensorE and
fp32 PSUM accumulation; softmax runs in the transposed
"feature-on-partition" space so its reduction lands on the free dim):
  qT/kT per head-pair  [128, T]  = Wpair^T  x  X^T
  v    per s-tile      [128, 8*65] = X^T^T  x  Wv_all (65th col set to 1)
  ST block [s=128, t=512] = kT_slice^T @ qT_slice   (row-packed head pairs)
  expST = exp(ST/8) (ScalarE), causal-masked via gpsimd affine_select
  avT [65, 512] += [V|1]^T @ expST   -> rows 0:64 = (A@V)^T, row 64 = sums
  normalize via 1/sums broadcast (rank-1 fp32 matmul) and write concatT
  partial = concatT^T @ WoST  (accumulated over 4 feature chunks)
"""

import os
import sys

for _p in ("/opt/trn_rl_repo", "/root/.axon_site/_ro/trn_rl_repo"):
    if os.path.isdir(_p) and _p not in sys.path:
        sys.path.append(_p)

import numpy as np

import concourse.mybir as mybir
import concourse.tile as tile
from concourse import bacc

B, T, D, H, K = 4, 2048, 1024, 16, 64
HG = 8          # heads per core
NPAIR = 4       # head pairs per core
P = 128
DC = D // P     # 8 contraction chunks for the projections
NS = T // P     # 16 key tiles
NT = T // 512   # 4 query tiles of 512
F32 = mybir.dt.float32
F16 = mybir.dt.float16


def build_module():
    nc = bacc.Bacc("TRN2")
    XT = nc.dram_tensor("xt", [D, T], F16, kind="ExternalInput").ap()
    WQ = nc.dram_tensor("wq", [NPAIR, D, P], F16, kind="ExternalInput").ap()
    WK = nc.dram_tensor("wk", [NPAIR, D, P], F16, kind="ExternalInput").ap()
    WV = nc.dram_tensor("wv", [D, HG * K], F16, kind="ExternalInput").ap()
    WO = nc.dram_tensor("wo", [HG * K, D], F16, kind="ExternalInput").ap()
    OUT = nc.dram_tensor("out", [T, D], F16, kind="ExternalOutput").ap()

    with tile.TileContext(nc) as tc:
        with tc.tile_pool(name="persist", bufs=1) as pp:
            xt_sb = pp.tile([P, DC, T], F16)            # X^T, 32 KB/partition
            v_sb = pp.tile([P, NS, HG * (K + 1)], F16)  # V + ones col per head
            concat_sb = pp.tile([P, NPAIR, T], F16)     # concat(heads)^T
            tri_sb = pp.tile([P, P], F16)   # causal triangle: 1 where x >= p

            xt_r = XT.rearrange("(c p) t -> c p t", p=P)
            for c in range(DC):
                nc.sync.dma_start(out=xt_sb[:, c, :], in_=xt_r[c])
            wo_sb = pp.tile([P, NPAIR, D], F16)
            nc.scalar.dma_start(
                out=wo_sb, in_=WO.rearrange("(s p) o -> p s o", p=P)
            )
            nc.vector.memset(tri_sb, 1.0)
            nc.gpsimd.affine_select(
                out=tri_sb,
                in_=tri_sb,
                compare_op=mybir.AluOpType.is_ge,
                fill=0.0,
                base=0,
                channel_multiplier=-1,
                pattern=[[1, P]],
            )
            # ones column (index 64 of each head's 65-wide slot)
            v_slots = v_sb.rearrange("p s (h x) -> p s h x", x=K + 1)
            nc.vector.memset(v_slots[:, :, :, K : K + 1], 1.0)

            # ---- V + Q/K projections + attention, software-pipelined ----
            # Projection matmuls for V (tail) and the NEXT pair's Q/K are
            # interleaved into the attention loop so the PE fills the
            # stalls where it would otherwise wait on ScalarE's exp.
            with (
                tc.tile_pool(name="wvp", bufs=1) as wvp,
                tc.tile_pool(name="attn", bufs=1) as ap_,
                tc.tile_pool(name="psa", bufs=1, space="PSUM") as psa,
            ):
                wv_sb = wvp.tile([P, DC, HG * K], F16)
                wv_r = WV.rearrange("(c p) n -> c p n", p=P)
                for c in range(DC):
                    nc.scalar.dma_start(out=wv_sb[:, c, :], in_=wv_r[c])

                def v_group_ops(s):
                    ps = psa.tile(
                        [P, HG * K], F32, tag="mm", bufs=2, name=f"vps{s}"
                    )
                    ops = [
                        (
                            lambda c=c, ps=ps, s=s: nc.tensor.matmul(
                                ps,
                                xt_sb[:, c, s * P : (s + 1) * P],
                                wv_sb[:, c, :],
                                start=(c == 0),
                                stop=(c == DC - 1),
                            )
                        )
                        for c in range(DC)
                    ]
                    ops.append(
                        lambda ps=ps, s=s: nc.vector.tensor_copy(
                            v_slots[:, s, :, 0:K],
                            ps.rearrange("p (h k) -> p h k", k=K),
                        )
                    )
                    return ops

                def proj_pair(pr):
                    """DMA the pair's weights; return (q, k, deferred ops)."""
                    wq_sb = ap_.tile(
                        [P, DC, P], F16, tag="wq", bufs=2, name=f"wq{pr}"
                    )
                    wk_sb = ap_.tile(
                        [P, DC, P], F16, tag="wk", bufs=2, name=f"wk{pr}"
                    )
                    nc.scalar.dma_start(
                        out=wq_sb, in_=WQ[pr].rearrange("(c p) m -> p c m", p=P)
                    )
                    nc.scalar.dma_start(
                        out=wk_sb, in_=WK[pr].rearrange("(c p) m -> p c m", p=P)
                    )
                    q_sb = ap_.tile([P, T], F16, tag="q", bufs=2, name=f"q{pr}")
                    k_sb = ap_.tile([P, T], F16, tag="k", bufs=2, name=f"k{pr}")
                    ops = []
                    for w_sb, qk_sb, nm in (
                        (wq_sb, q_sb, "q"),
                        (wk_sb, k_sb, "k"),
                    ):
                        for tt in range(NT):
                            ps = psa.tile(
                                [P, 512], F32, tag="mm", bufs=2,
                                name=f"{nm}ps{pr}_{tt}",
                            )
                            for c in range(DC):
                                ops.append(
                                    lambda ps=ps, w_sb=w_sb, c=c, tt=tt:
                                    nc.tensor.matmul(
                                        ps,
                                        w_sb[:, c, :],
                                        xt_sb[
                                            :, c, tt * 512 : (tt + 1) * 512
                                        ],
                                        start=(c == 0),
                                        stop=(c == DC - 1),
                                    )
                                )
                            ops.append(
                                lambda ps=ps, qk_sb=qk_sb, tt=tt:
                                nc.vector.tensor_copy(
                                    qk_sb[:, tt * 512 : (tt + 1) * 512], ps
                                )
                            )
                    return q_sb, k_sb, ops

                # upfront: V for the first four key tiles + pair 0's Q/K
                for s in range(4):
                    for op in v_group_ops(s):
                        op()
                q_sb, k_sb, ops0 = proj_pair(0)
                for op in ops0:
                    op()
                vqueue = [op for s in range(4, NS) for op in v_group_ops(s)]
                pending = []
                v_done = [0]

                for pr in range(NPAIR):
                    if pr < NPAIR - 1:
                        nq_sb, nk_sb, nops = proj_pair(pr + 1)
                        pending.extend(nops)

                    for tt in range(NT):
                        if pr == 0:
                            # V for this tt's key tiles must be in flight
                            need = (4 * tt) * 9
                            while v_done[0] < need and vqueue:
                                vqueue.pop(0)()
                                v_done[0] += 1
                        avs = [
                            psa.tile(
                                [K + 1, 512], F32, tag="av", bufs=2,
                                name=f"av{pr}_{tt}_{h2}",
                            )
                            for h2 in range(2)
                        ]
                        n_s = 4 * tt + 4
                        for si in range(n_s):
                            for _ in range(4):
                                if vqueue:
                                    vqueue.pop(0)()
                                    v_done[0] += 1
                                elif pending:
                                    pending.pop(0)()
                            # diagonal blocks: only cols >= 128*m can be valid
                            m = si - 4 * tt
                            off = max(m, 0) * P
                            nv = 512 - off
                            # both heads' score blocks in one 2-bank tile
                            st = psa.tile([P, 2, 512], F32, tag="stw", bufs=2)
                            ex = ap_.tile([P, 2, 512], F16, tag="exp", bufs=6)
                            for h in range(2):
                                lo, hi = h * K, (h + 1) * K
                                nc.tensor.matmul(
                                    st[:, h, 0:nv],
                                    k_sb[lo:hi, si * P : (si + 1) * P],
                                    q_sb[
                                        lo:hi,
                                        tt * 512 + off : (tt + 1) * 512,
                                    ],
                                    start=True,
                                    stop=True,
                                    tile_position=(lo, 0),
                                )
                            nc.scalar.activation(
                                ex[:, :, 0:nv], st[:, :, 0:nv],
                                mybir.ActivationFunctionType.Exp,
                                scale=0.125,
                            )
                            if m >= 0:  # mask both heads' leading triangles
                                nc.vector.tensor_mul(
                                    ex[:, :, 0:P],
                                    ex[:, :, 0:P],
                                    tri_sb.unsqueeze(1).broadcast_to(
                                        [P, 2, P]
                                    ),
                                )
                            for h in range(2):
                                slot = (2 * pr + h) * (K + 1)
                                nc.tensor.matmul(
                                    avs[h][:, off:512],
                                    v_sb[:, si, slot : slot + K + 1],
                                    ex[:, h, 0:nv],
                                    start=(si == 0),
                                    stop=(si == n_s - 1),
                                )
                        # Free the AV accumulators fast: copy raw (A@V)^T and
                        # the sums row out, then normalize in place off the
                        # critical path (divide happens before outproj reads).
                        for h in range(2):
                            cols = slice(tt * 512, (tt + 1) * 512)
                            sums = ap_.tile([1, 512], F32, tag="sums", bufs=4)
                            nc.vector.tensor_copy(sums, avs[h][K : K + 1, :])
                            if h == 0:
                                dst = concat_sb[0:K, pr, cols]
                                nc.vector.tensor_copy(dst, avs[h][0:K, :])
                            else:
                                dst = ap_.tile([K, 512], F16, tag="tmpb", bufs=4)
                                nc.vector.tensor_copy(dst, avs[h][0:K, :])
                            recip = ap_.tile([1, 512], F32, tag="recip", bufs=4)
                            nc.vector.reciprocal_approx_fast(recip, sums)
                            bc_sb = ap_.tile([K, 512], F32, tag="bc_sb", bufs=4)
                            nc.gpsimd.partition_broadcast(bc_sb, recip)
                            nc.vector.tensor_mul(dst, dst, bc_sb)
                            if h == 1:
                                # partition-shifted write via DMA bounce
                                nc.sync.dma_start(
                                    out=concat_sb[K:P, pr, cols], in_=dst
                                )

                    # next pair's projections must be complete before its
                    # attention starts; flush whatever wasn't interleaved
                    while pending:
                        pending.pop(0)()
                    if pr < NPAIR - 1:
                        q_sb, k_sb = nq_sb, nk_sb

            # ---- output projection (partial: this core's 512 features) ----
            with (
                tc.tile_pool(name="op", bufs=1) as op_,
                tc.tile_pool(name="pso", bufs=1, space="PSUM") as pso,
            ):
                for t16 in range(T // P):
                    for oc in range(2):
                        ps = pso.tile([P, 512], F32, tag="mm", bufs=4)
                        for s4 in range(NPAIR):
                            nc.tensor.matmul(
                                ps,
                                concat_sb[:, s4, t16 * P : (t16 + 1) * P],
                                wo_sb[:, s4, oc * 512 : (oc + 1) * 512],
                                start=(s4 == 0),
                                stop=(s4 == NPAIR - 1),
                            )
                        st_o = op_.tile([P, 512], F16, tag="outst", bufs=4)
                        nc.vector.tensor_copy(st_o, ps)
                        nc.sync.dma_start(
                            out=OUT[
                                t16 * P : (t16 + 1) * P, oc * 512 : (oc + 1) * 512
                            ],
                            in_=st_o,
                        )
    nc.compile()
    return nc


def shard_inputs(X, Wq, Wk, Wv, Wo):
    """Host-side shard prep: core c handles batch c//2, head group c%2."""
    in_maps = []
    for c in range(8):
        b, g = c // 2, c % 2
        heads = range(g * HG, (g + 1) * HG)
        wq = np.stack(
            [
                np.concatenate([Wq[g * HG + 2 * p], Wq[g * HG + 2 * p + 1]], axis=1)
                for p in range(NPAIR)
            ]
        )
        wk = np.stack(
            [
                np.concatenate([Wk[g * HG + 2 * p], Wk[g * HG + 2 * p + 1]], axis=1)
                for p in range(NPAIR)
            ]
        )
        wv = np.concatenate([Wv[h] for h in heads], axis=1)
        wo = Wo[:, g * 512 : (g + 1) * 512].T
        in_maps.append(
            {
                "xt": np.ascontiguousarray(X[b].T).astype(np.float16),
                "wq": np.ascontiguousarray(wq).astype(np.float16),
                "wk": np.ascontiguousarray(wk).astype(np.float16),
                "wv": np.ascontiguousarray(wv).astype(np.float16),
                "wo": np.ascontiguousarray(wo).astype(np.float16),
            }
        )
    return in_maps


_MODULE = None


def _get_module():
    global _MODULE
    if _MODULE is None:
        _MODULE = build_module()
    return _MODULE


def kernel(X, Wq, Wk, Wv, Wo, bo, _want_results=None):
    from concourse.bass_utils import run_bass_kernel_spmd

    nc = _get_module()
    in_maps = shard_inputs(
        np.asarray(X), np.asarray(Wq), np.asarray(Wk), np.asarray(Wv), np.asarray(Wo)
    )
    res = run_bass_kernel_spmd(nc, in_maps, core_ids=list(range(8)))
    if _want_results is not None:
        _want_results.append(res)
    out = np.empty((B, T, H * K), dtype=np.float32)
    bo = np.asarray(bo, dtype=np.float32)
    for b in range(B):
        out[b] = (
            res.results[2 * b]["out"].astype(np.float32)
            + res.results[2 * b + 1]["out"].astype(np.float32)
            + bo
        )
    return out
